# revision 12
# baseline (speedup 1.0000x reference)
"""Bidirectional Mamba selective scan on 8 Trainium2 NeuronCores.

Sharding: core c -> (batch b = c//2, d_inner half = c%2). Each core receives
x[b] pre-transposed to [D, L] with its own d-half rows first, computes the
(replicated, small) x_proj and dt_proj matmuls locally, and runs both scan
directions fully on-core: zero cross-core communication, one SPMD NEFF.

Per-core dataflow (Lc-chunked along L, d on partitions, s-major state dim),
engine-balanced so the DVE runs little besides the serial scans:
  PE : dbc = x_proj_w @ x (bf16); delta matmul (K=32, f32); the s-summation
       y = sum_s h_s*C_s as 16 identity-matmuls accumulating in PSUM; the
       (D+D_b)*x term as a diagonal-weight matmul; fwd/bwd combine by
       re-feeding y_acc through an identity matmul.
  ACT: softplus via Exp/Ln (one table set); 16x exp(delta*A_s) per tile;
       PSUM->SBUF copies and bf16 conversions.
  DVE: 16 tensor_tensor_scan per tile (the serial recurrence, ~2.3cyc/elem)
       + 16 bf16 Cmult (2x mode) + 2 of the bx multiplies.
  GPS: u = delta*x and 14 of the 16 bx = u*B_s multiplies (bf16).
  DMA: x / weight loads, B/C partition-broadcast via DRAM bounce (bf16),
       y stores.
Chunk i+1's head (loads, dbc/delta matmuls, broadcasts, softplus) is
emitted before chunk i's body so every engine stays fed across chunk
boundaries; the scan carry is read straight out of the previous chunk's h
tile (no state copies).
"""

import numpy as np
import ml_dtypes

import bass_rust
import concourse.bass as bass
import concourse.mybir as mybir
import concourse.tile as tile
from concourse.bass_utils import run_bass_kernel_spmd
from concourse.vector_clock import ScopedClock

F32 = mybir.dt.float32
# 2-byte dtype for the DVE 2x paths: fp16 (10 mantissa bits) rather than
# bf16 -- same speed class, ~8x tighter rounding for the scan operands.
BF16 = mybir.dt.float16
NPBF16 = np.float16
DA_HALF = True        # dA in fp16 (tests whether the scan gets a 2x uop)
OP = mybir.AluOpType
AF = mybir.ActivationFunctionType

B, L, DI, S, R = 4, 2048, 1024, 16, 32
DH = DI // 2          # d channels per core
NK = DI // 128        # K-chunks for the dbc matmul
NT = DH // 128        # d-tiles per core
LC = 512              # L chunk
NCH = L // LC
NB = R + 2 * S        # dbc rows (64)
BX_DVE = 2            # bx multiplies done on DVE (rest on gpsimd)

# dtype knobs (bisectable): False -> float32 on that path
XS_BF16 = True        # x tiles + dbc weights + ddiag matmul operands
BC_BF16 = True        # broadcast B/C rows
HP_BF16 = True        # h scan output, Cmult products, ident, y_acc, state
UX_BF16 = True        # u and bx (gpsimd operands)


def _dt(flag):
    return BF16 if flag else F32


def _npdt(flag):
    return NPBF16 if flag else np.float32


class SplitDrainTileContext(tile.TileContext):
    """TileContext whose exit drain splits sem waits across instructions.

    This walrus build rejects instructions carrying >2 sync-wait commands
    ("Too many sync wait commands" in CoreV3 codegen). Stock TileContext
    attaches one wait per outstanding proc to the single final SP drain;
    emit one wait-carrier nop per proc instead.
    """

    def _drain_and_barrier(self, tick_clock, wait_clock):
        ticks = list(tick_clock.global_clock)
        self.nc.sync.drain()
        for i, t in enumerate(ticks):
            if t > 0:
                partial = bass_rust.VectorClock(
                    [t if j == i else 0 for j in range(len(ticks))]
                )
                carrier = self.nc.sync.nop(nofuse=True, hint="split_drain_wait")
                wait_clock.add_sem_waits(carrier.ins, ScopedClock({None: partial}))

        self.nc.all_engine_barrier()
        assert self.sems is not None
        popped = self.nc._tile_sem_poison_stack.pop()
        assert popped is self._sem_poison
        self.nc.clear_and_free_semaphores(list(self.sems.allocated().values()))
        self.nc.all_engine_barrier()


MAX_WAITS = 1  # sync-wait commands this walrus accepts per instruction


def legalize_sync_waits(json_bytes):
    """Split >cap on_wait conditions onto EventSemaphore carriers.

    This walrus build errors with "Too many sync wait commands" when one
    instruction carries more than `cap` waits. Hoist the excess onto
    same-engine EventSemaphore instructions inserted just before; engine
    program order makes the waits still happen-before the instruction
    (for DMAs: before descriptor enqueue).
    """
    import json

    m = json.loads(json_bytes)
    for f in m["functions"]:
        for bb in f["blocks"]:
            out = []
            changed = False
            for inst in bb["instructions"]:
                si = inst.get("sync_info") or {}
                ws = si.get("on_wait") or []
                cap = MAX_WAITS
                if len(ws) > cap:
                    changed = True
                    keep = ws[:cap]
                    rest = ws[cap:]
                    for i in range(0, len(rest), cap):
                        out.append({
                            "debug": inst.get("debug", 0),
                            "engine": inst["engine"],
                            "ins": [],
                            "name": f"{inst['name']}_w{i}",
                            "opcode": "EventSemaphore",
                            "outs": [],
                            "sync_info": {
                                "on_update": [],
                                "on_wait": rest[i:i + cap],
                            },
                        })
                    si["on_wait"] = keep
                    inst["sync_info"] = si
                out.append(inst)
            if changed:
                bb["instructions"] = out
    return json.dumps(m).encode()


def _bcast_ap(row_ap, parts=128):
    """View a single-partition row AP as a partition-stride-0 broadcast."""
    return bass.AP(
        tensor=row_ap.tensor,
        offset=row_ap.offset,
        ap=[[0, parts]] + [list(d) for d in row_ap.ap[1:]],
    )


def _rep_ap(ap2d, times):
    """Replicate a [P, N] AP along a new middle free dim via stride 0."""
    return bass.AP(
        tensor=ap2d.tensor,
        offset=ap2d.offset,
        ap=[list(ap2d.ap[0]), [0, times]] + [list(d) for d in ap2d.ap[1:]],
    )


def build_nc(repeat=1):
    nc = bass.Bass()

    xT = nc.dram_tensor("xT", [DI, L], _dt(XS_BF16), kind="ExternalInput")
    wdbc = nc.dram_tensor("wdbc", [2, DI, NB], _dt(XS_BF16), kind="ExternalInput")
    wdt = nc.dram_tensor("wdt", [2, R, DH], F32, kind="ExternalInput")
    bdt = nc.dram_tensor("bdt", [2, DH, 1], F32, kind="ExternalInput")
    Adr = nc.dram_tensor("A", [2, DH, S], F32, kind="ExternalInput")
    ident = nc.dram_tensor("ident", [128, 128], _dt(HP_BF16), kind="ExternalInput")
    ddiag = nc.dram_tensor("ddiag", [NT, 128, 128], _dt(XS_BF16),
                           kind="ExternalInput")
    yT = nc.dram_tensor("yT", [DH, L], F32, kind="ExternalOutput")

    from contextlib import ExitStack

    with SplitDrainTileContext(nc) as tc:
        with ExitStack() as stack:
            pool_specs = {
                "persist": dict(bufs=1),
                "xs": dict(bufs=2),
                "dbc_ps": dict(bufs=2, space="PSUM"),
                "delta_ps": dict(bufs=2, space="PSUM"),
                "ypsum": dict(bufs=2, space="PSUM"),
                "dbc": dict(bufs=2),
                "bc16": dict(bufs=2),
                "ez": dict(bufs=2),
                "delta": dict(bufs=6),
                "u": dict(bufs=2),
                "bcb": dict(bufs=2),
                "bcc": dict(bufs=2),
                "da": dict(bufs=3),
                "bxall": dict(bufs=1),
                "h": dict(bufs=1),
                "pall": dict(bufs=1),
                "yst": dict(bufs=1),
                "bcd": dict(bufs=2, space="DRAM"),
            }
            pools = {
                name: stack.enter_context(tc.tile_pool(name=name, **kw))
                for name, kw in pool_specs.items()
            }
            persist = pools["persist"]
            xs_pool = pools["xs"]
            dbc_ps = pools["dbc_ps"]
            delta_ps = pools["delta_ps"]
            ypsum_pool = pools["ypsum"]
            dbc_pool = pools["dbc"]
            bc16_pool = pools["bc16"]
            ez_pool = pools["ez"]
            delta_pool = pools["delta"]
            u_pool = pools["u"]
            bcb_pool = pools["bcb"]
            bcc_pool = pools["bcc"]
            da_pool = pools["da"]
            bxall_pool = pools["bxall"]
            h_pool = pools["h"]
            pall_pool = pools["pall"]
            yst_pool = pools["yst"]
            bcd_pool = pools["bcd"]
            # ---- persistent loads ----
            wdbc_sb = [[None] * NK for _ in range(2)]
            wdt_sb = [None] * 2
            bdt_sb = [[None] * NT for _ in range(2)]
            A_sb = [[None] * NT for _ in range(2)]
            for d in range(2):
                for k in range(NK):
                    w = persist.tile([128, NB], _dt(XS_BF16), tag=f"wdbc{d}_{k}")
                    nc.sync.dma_start(out=w[:, :], in_=wdbc[d, k * 128:(k + 1) * 128, :])
                    wdbc_sb[d][k] = w
                wt = persist.tile([R, DH], F32, tag=f"wdt{d}")
                nc.sync.dma_start(out=wt[:, :], in_=wdt[d, :, :])
                wdt_sb[d] = wt
                for t in range(NT):
                    bb = persist.tile([128, 1], F32, tag=f"bdt{d}_{t}")
                    nc.sync.dma_start(out=bb[:, :], in_=bdt[d, t * 128:(t + 1) * 128, :])
                    bdt_sb[d][t] = bb
                    aa = persist.tile([128, S], F32, tag=f"A{d}_{t}")
                    nc.sync.dma_start(out=aa[:, :], in_=Adr[d, t * 128:(t + 1) * 128, :])
                    A_sb[d][t] = aa

            ident_sb = persist.tile([128, 128], _dt(HP_BF16), tag="ident")
            nc.sync.dma_start(out=ident_sb[:, :], in_=ident[:, :])
            ddiag_sb = []
            for t in range(NT):
                dd = persist.tile([128, 128], _dt(XS_BF16), tag=f"ddiag{t}")
                nc.sync.dma_start(out=dd[:, :], in_=ddiag[t, :, :])
                ddiag_sb.append(dd)

            y_acc = [persist.tile([128, L], _dt(HP_BF16), tag=f"yacc{t}", name=f"yacc{t}")
                     for t in range(NT)]
            state_sb = [persist.tile([128, S], _dt(HP_BF16), tag=f"state{t}",
                                     name=f"state{t}") for t in range(NT)]

            staged = {}

            def head(d, ci):
                lsl = slice(ci * LC, (ci + 1) * LC)
                # stream this chunk's full-D xT columns (own half = 0..3)
                xs = xs_pool.tile([128, NK, LC], _dt(XS_BF16))
                for k in range(NK):
                    nc.sync.dma_start(
                        out=xs[:, k, :], in_=xT[k * 128:(k + 1) * 128, lsl]
                    )
                # dbc = x_proj_w @ x : [64, LC]
                ps = dbc_ps.tile([NB, LC], F32)
                for k in range(NK):
                    nc.tensor.matmul(
                        ps[:, :], wdbc_sb[d][k][:, :], xs[:, k, :],
                        start=(k == 0), stop=(k == NK - 1),
                    )
                dbc_sb = dbc_pool.tile([NB, LC], F32)
                nc.scalar.copy(out=dbc_sb[:, :], in_=ps[:, :])
                # B and C rows -> bf16, bounce through DRAM for the
                # partition broadcast (stride-0 partition APs are DRAM-only)
                bc16 = bc16_pool.tile([2 * S, LC], _dt(BC_BF16))
                nc.scalar.copy(out=bc16[:, :], in_=dbc_sb[R:NB, :])
                bc_dram = bcd_pool.tile([2 * S, LC], _dt(BC_BF16), tag="bcd", name="bc_dram")
                nc.sync.dma_start(out=bc_dram[:, :], in_=bc16[:, :])
                bc_b = bcb_pool.tile([128, S, LC], _dt(BC_BF16))
                bc_c = bcc_pool.tile([128, S, LC], _dt(BC_BF16))
                for s in range(S):
                    nc.sync.dma_start(
                        out=bc_b[:, s, :], in_=_bcast_ap(bc_dram[s:s + 1, :])
                    )
                    nc.sync.dma_start(
                        out=bc_c[:, s, :], in_=_bcast_ap(bc_dram[S + s:S + s + 1, :])
                    )
                # delta per tile: softplus(dt_w @ dbc_delta + bias)
                deltas = []
                for t in range(NT):
                    dps = delta_ps.tile([128, LC], F32)
                    nc.tensor.matmul(
                        dps[:, :], wdt_sb[d][:, t * 128:(t + 1) * 128],
                        dbc_sb[0:R, :], start=True, stop=True,
                    )
                    # softplus(z) = ln(exp(z) + 1); Exp and Ln share one ACT
                    # func set (natural_log_exp_and_others) => no table switch
                    ez = ez_pool.tile([128, LC], F32)
                    nc.scalar.activation(
                        out=ez[:, :], in_=dps[:, :], func=AF.Exp,
                        bias=bdt_sb[d][t][:, :], scale=1.0,
                    )
                    delta = delta_pool.tile([128, LC], F32)
                    nc.scalar.activation(
                        out=delta[:, :], in_=ez[:, :], func=AF.Ln,
                        bias=1.0, scale=1.0,
                    )
                    deltas.append(delta)
                staged[(d, ci)] = (xs, bc_b, bc_c, deltas)

            def body(d, ci, first_of_dir):
                xs, bc_b, bc_c, deltas = staged.pop((d, ci))
                fwd = d == 0
                lsl = slice(ci * LC, (ci + 1) * LC)
                for t in range(NT):
                    delta = deltas[t]
                    # u = delta * x (own-half rows are xs tiles 0..3)
                    u = u_pool.tile([128, LC], _dt(UX_BF16))
                    nc.vector.tensor_tensor(
                        out=u[:, :], in0=delta[:, :], in1=xs[:, t, :],
                        op=OP.mult,
                    )
                    # bx_s = u * B_s for all s in one op (stride-0 repeat)
                    bxall = bxall_pool.tile([128, S, LC], _dt(UX_BF16))
                    nc.vector.tensor_tensor(
                        out=bxall[:, :, :], in0=_rep_ap(u[:, :], S),
                        in1=bc_b[:, :, :], op=OP.mult,
                    )
                    # dA_s = exp(delta * A_s) on ACT (da pool paces run-ahead)
                    da_tiles = []
                    for s in range(S):
                        da = da_pool.tile([128, LC], BF16 if DA_HALF else F32)
                        nc.scalar.activation(
                            out=da[:, :], in_=delta[:, :], func=AF.Exp,
                            scale=A_sb[d][t][:, s:s + 1],
                        )
                        da_tiles.append(da)

                    h = h_pool.tile([128, S, LC], _dt(HP_BF16))
                    for s in range(S):
                        if first_of_dir:
                            init = 0.0
                        else:
                            init = state_sb[t][:, s:s + 1]
                        if fwd:
                            nc.vector.tensor_tensor_scan(
                                out=h[:, s, :], data0=da_tiles[s][:, :],
                                data1=bxall[:, s, :], initial=init,
                                op0=OP.mult, op1=OP.add,
                            )
                        else:
                            nc.vector.tensor_tensor_scan(
                                out=h[:, s, :][:, ::-1],
                                data0=da_tiles[s][:, :][:, ::-1],
                                data1=bxall[:, s, :][:, ::-1], initial=init,
                                op0=OP.mult, op1=OP.add,
                            )
                    # save carry state (ACT; h must be read before the next
                    # chunk's scans reuse the single h buffer)
                    col = LC - 1 if fwd else 0
                    nc.scalar.copy(out=state_sb[t][:, :], in_=h[:, :, col])
                    # p_s = h_s * C_s for all s in one op, then PE-accumulate
                    pall = pall_pool.tile([128, S, LC], _dt(HP_BF16))
                    nc.vector.tensor_tensor(
                        out=pall[:, :, :], in0=h[:, :, :], in1=bc_c[:, :, :],
                        op=OP.mult,
                    )
                    ypsum = ypsum_pool.tile([128, LC], F32)
                    for s in range(S):
                        nc.tensor.matmul(
                            ypsum[:, :], ident_sb[:, :], pall[:, s, :],
                            start=(s == 0), stop=False,
                        )

                    if fwd:
                        # + (D + D_b) * x, then stage to y_acc (bf16)
                        nc.tensor.matmul(
                            ypsum[:, :], ddiag_sb[t][:, :], xs[:, t, :],
                            start=False, stop=True,
                        )
                        nc.scalar.copy(out=y_acc[t][:, lsl], in_=ypsum[:, :])
                    else:
                        # + fwd partial, then emit the finished chunk
                        nc.tensor.matmul(
                            ypsum[:, :], ident_sb[:, :], y_acc[t][:, lsl],
                            start=False, stop=True,
                        )
                        yst = yst_pool.tile([128, LC], F32)
                        nc.scalar.copy(out=yst[:, :], in_=ypsum[:, :])
                        nc.sync.dma_start(
                            out=yT[t * 128:(t + 1) * 128, lsl], in_=yst[:, :]
                        )

            # ---- main loop: software-pipelined heads ----
            for _rep in range(repeat):
                combos = [(0, ci) for ci in range(NCH)]
                combos += [(1, ci) for ci in range(NCH - 1, -1, -1)]
                for i, (d, ci) in enumerate(combos):
                    if i == 0:
                        head(d, ci)
                    if i + 1 < len(combos):
                        head(*combos[i + 1])
                    body(d, ci, first_of_dir=(ci == (0 if d == 0 else NCH - 1)))

    return nc


_NC_CACHE = []
TRACE = False
LAST_EXEC_NS = None
LAST_RESULTS = None


def _get_nc():
    if not _NC_CACHE:
        nc = build_nc()
        legal = legalize_sync_waits(nc.to_json_bytes())
        nc.to_json_bytes = lambda: legal
        _NC_CACHE.append(nc)
    return _NC_CACHE[0]


def kernel(x, x_proj_w, dt_proj_w, dt_proj_b, A_log, D,
           x_proj_b_w, dt_proj_b_w, dt_proj_b_b, A_b_log, D_b):
    x = np.asarray(x, np.float32)
    wdbc_full = np.stack(
        [np.asarray(x_proj_w, np.float32).T, np.asarray(x_proj_b_w, np.float32).T]
    ).astype(_npdt(XS_BF16))  # [2, DI, 64]
    wdt_full = np.stack(
        [np.asarray(dt_proj_w, np.float32).T, np.asarray(dt_proj_b_w, np.float32).T]
    )  # [2, R, DI]
    bdt_full = np.stack(
        [np.asarray(dt_proj_b, np.float32), np.asarray(dt_proj_b_b, np.float32)]
    )  # [2, DI]
    A_full = np.stack(
        [-np.exp(np.asarray(A_log, np.float32)),
         -np.exp(np.asarray(A_b_log, np.float32))]
    )  # [2, DI, S]
    dsum_full = np.asarray(D, np.float32) + np.asarray(D_b, np.float32)
    ident_np = np.eye(128, dtype=_npdt(HP_BF16))

    # Per half: permute d so the core's own half comes first; the dbc
    # matmul contracts over all of d, so weights get the same row permute.
    perm = [np.r_[0:DI], np.r_[DH:DI, 0:DH]]
    in_maps = []
    half_common = []
    for half in range(2):
        p = perm[half]
        ds = half * DH
        dsum_h = dsum_full[ds:ds + DH]
        ddiag_np = np.zeros((NT, 128, 128), np.float32)
        for t in range(NT):
            ddiag_np[t] = np.diag(dsum_h[t * 128:(t + 1) * 128])
        half_common.append({
            "wdbc": np.ascontiguousarray(wdbc_full[:, p, :]),
            "wdt": np.ascontiguousarray(wdt_full[:, :, ds:ds + DH]),
            "bdt": np.ascontiguousarray(bdt_full[:, ds:ds + DH, None]),
            "A": np.ascontiguousarray(A_full[:, ds:ds + DH, :]),
            "ident": ident_np,
            "ddiag": ddiag_np.astype(_npdt(XS_BF16)),
        })
    for c in range(8):
        b, half = c // 2, c % 2
        xTb = np.ascontiguousarray(x[b].T[perm[half], :]).astype(_npdt(XS_BF16))
        in_maps.append(dict(half_common[half], xT=xTb))

    nc = _get_nc()
    global LAST_EXEC_NS, LAST_RESULTS
    res = run_bass_kernel_spmd(
        nc, in_maps, core_ids=list(range(8)), trace=TRACE,
        trace_cores=list(range(8)) if TRACE else None,
    )
    LAST_EXEC_NS = res.exec_time_ns
    LAST_RESULTS = res

    y = np.empty((B, L, DI), np.float32)
    for c in range(8):
        b, half = c // 2, c % 2
        ds = half * DH
        y[b, :, ds:ds + DH] = res.results[c]["yT"].T
    return y
